# revision 1
# baseline (speedup 1.0000x reference)
"""Causal self-attention Trainium2 Bass kernel.

Problem: B=4, T=2048, D=1024, H=16, head_dim=64.
Sharding: 8 cores = (batch b in 0..3) x (head-group g in 0..1, 8 heads each).
Each core computes a partial projection output for its batch over its 512
model dims; the host sums the two partials per batch (b_proj is fed to the
g==0 core only).

Kernel structure (per core):
- Phase A: qkv projections in float32r (full PE rate at N=512); q^T/k^T
  kept resident as [feat, t] tiles, v natural-layout with a ones column
  appended per head (so attn @ v' also yields the softmax denominator).
- Phase B: attention, q-chunk (512 cols) OUTER, key-tile (128) inner.
  Transposed scores [keys, q] go PSUM -> exp (ACT, scale=1/sqrt(hd)) ->
  bf16 esb tiles; causal diagonal blocks are masked by one strided
  tensor_mul per pair; attn @ v' accumulates in a 1-bank PSUM tile.
  Small PSUM tiles (2-banks scores x3 bufs, 1-bank y x2) keep PE and ACT
  streaming concurrently; the qi-outer order lets phase C overlap B.
- Phase C: output projection from resident y^T tiles, bias fused into the
  PSUM evacuation, 2-row-group batched output DMAs.

Timing note: `reps` is implemented as a device-side `tc.For_i` loop (the
body is idempotent, so executing it R times yields the same output).  A
repeat-delta between reps=1 and reps=R therefore measures the marginal
on-device execution time of one body iteration, with the fixed per-call
axon dispatch + NEFF transfer/load overhead cancelled.  Emitting the body
R times unrolled (as earlier revisions did) makes the NEFF size scale
with R, so that delta is dominated by per-instruction NEFF load/transfer
overhead (~100us/emitted instruction) rather than device execution.
"""

import numpy as np

import concourse.bacc as bacc
import concourse.bass as bass
import concourse.mybir as mybir
import concourse.tile as tile
from concourse.bass_utils import run_bass_kernel_spmd

F32 = mybir.dt.float32
F32R = mybir.dt.float32r
BF16 = mybir.dt.bfloat16
AF = mybir.ActivationFunctionType

B, T, D, H = 4, 2048, 1024, 16
HD = 64              # head dim
HPC = 8              # heads per core
DC = HPC * HD        # 512 model dims per core
SCALE = 1.0 / np.sqrt(HD)

_NC_CACHE = {}


def build_nc(t=T, reps=1, phases="ABC", no_mask=False, no_norm=False,
             no_exp=False):
    """Build the single-core SPMD program. t = sequence length (for small
    sims). reps>1 repeats the computation via a device-side For_i loop
    (device-time measurement); phases/no_* are timing-ablation knobs
    (wrong numerics when used)."""
    nt = t // 128          # 128-row tiles over time
    nq = t // 512          # 512-col chunks over time
    KC = D // 128          # 8 contraction chunks for qkv
    MQK = DC // 128        # 4 feature tiles for each of q,k

    nc = bacc.Bacc("TRN2", target_bir_lowering=False, debug=False)

    xT_d = nc.dram_tensor("xT", [D, t], F32R, kind="ExternalInput")
    wq_d = nc.dram_tensor("wq", [D, DC], F32R, kind="ExternalInput")
    wk_d = nc.dram_tensor("wk", [D, DC], F32R, kind="ExternalInput")
    wv_d = nc.dram_tensor("wv", [D, DC], F32R, kind="ExternalInput")
    bq_d = nc.dram_tensor("bq", [1, DC], F32, kind="ExternalInput")
    bk_d = nc.dram_tensor("bk", [1, DC], F32, kind="ExternalInput")
    bv_d = nc.dram_tensor("bv", [1, DC], F32, kind="ExternalInput")
    wp_d = nc.dram_tensor("wp", [DC, D], F32R, kind="ExternalInput")
    bp_d = nc.dram_tensor("bp", [1, D], F32, kind="ExternalInput")
    ones_d = nc.dram_tensor("cones", [1, 512], F32R, kind="ExternalInput")
    out_d = nc.dram_tensor("out", [t, D], F32, kind="ExternalOutput")

    with tile.TileContext(nc) as tc:
      with tc.For_i(0, reps, 1) as _i:
        with tc.tile_pool(name="persist", bufs=1) as persist, \
             tc.tile_pool(name="vpool", bufs=1) as vpool, \
             tc.tile_pool(name="qkpool", bufs=1) as qkpool:

            # resident qk^T: [:, m, :] = q^T feats tile m, [:, 4+m, :] = k^T
            qkTb = qkpool.tile([128, 2 * MQK, t], F32R)

            # tmask[p, c] = 1 iff c >= p (keep lower triangle in [k, q])
            tmask = persist.tile([128, 128], BF16)
            nc.gpsimd.memset(tmask[:], 1.0)
            nc.gpsimd.affine_select(
                out=tmask[:], in_=tmask[:],
                compare_op=mybir.AluOpType.is_ge, fill=0.0,
                base=0, pattern=[[1, 128]], channel_multiplier=-1)
            # tmask2: 2 replicas so one strided mul masks a diagonal pair
            tmask2 = persist.tile([128, 2, 128], BF16)
            for r in range(2):
                nc.vector.tensor_copy(tmask2[:, r, :], tmask[:])

            ones_bc = persist.tile([128, nt * HPC], BF16)
            nc.gpsimd.dma_start(
                ones_bc[:], ones_d[0:1, 0:nt * HPC].to_broadcast([128, nt * HPC]))
            bv_bc = persist.tile([128, DC], F32)
            nc.gpsimd.dma_start(bv_bc[:], bv_d[0:1, :].to_broadcast([128, DC]))
            bp_bc = persist.tile([128, D], F32)
            nc.gpsimd.dma_start(bp_bc[:], bp_d[0:1, :].to_broadcast([128, D]))
            # partition-major per-feature-tile bias columns [128, MQK]
            bqp = persist.tile([128, MQK], F32)
            nc.sync.dma_start(bqp[:], bq_d.rearrange("o (m p) -> p (o m)", p=128))
            bkp = persist.tile([128, MQK], F32)
            nc.sync.dma_start(bkp[:], bk_d.rearrange("o (m p) -> p (o m)", p=128))

            # v' mega-tile (bf16): [128, nt, 8*65]; col h*65+64 holds ones
            vpm = vpool.tile([128, nt, HPC * (HD + 1)], BF16)
            nc.vector.tensor_copy(
                vpm.rearrange("p t (h e) -> p (t h) e", e=HD + 1)[:, :, HD:HD + 1],
                ones_bc[:].unsqueeze(2))

            # ---------------- Phase A: qkv ----------------
            with tc.tile_pool(name="phA_sb", bufs=1) as pa, \
                 tc.tile_pool(name="phA_w", bufs=2) as pw, \
                 tc.tile_pool(name="phA_ps", bufs=2, space="PSUM") as pps:

                # x^T resident: per-k-chunk DMAs so matmuls start early
                xTb = pa.tile([128, KC, t], F32R)
                for k in range(KC):
                    nc.sync.dma_start(
                        xTb[:, k, :], xT_d[k * 128:(k + 1) * 128, :])

                # q^T / k^T -> psum -> (bias-add) resident qkTb
                for sec, (w_d, b_s) in enumerate(
                        ((wq_d, bqp), (wk_d, bkp))
                        if ("q" in phases or "A" in phases) else ()):
                    ws = pw.tile([128, KC, DC], F32R, name=f"ws{sec}", tag="wsec")
                    nc.sync.dma_start(ws[:], w_d.rearrange("(k p) c -> p k c", p=128))
                    for m in range(MQK):
                        for np2 in range(nq // 2):
                            ps = pps.tile([128, 1024], F32, name="qkps", tag="psqk")
                            for k in range(KC):
                                for half in range(2):
                                    n = 2 * np2 + half
                                    nc.tensor.matmul(
                                        ps[:, half * 512:(half + 1) * 512],
                                        ws[:, k, m * 128:(m + 1) * 128],
                                        xTb[:, k, n * 512:(n + 1) * 512],
                                        start=(k == 0), stop=(k == KC - 1))
                            nc.vector.tensor_scalar_add(
                                qkTb[:, sec * MQK + m,
                                     np2 * 1024:(np2 + 1) * 1024],
                                ps[:], b_s[:, m:m + 1])

                # v natural (+bias) -> strided copy into v' tiles (bf16)
                if "v" in phases or "A" in phases:
                    wvs = pw.tile([128, KC, DC], F32R, name="wvs", tag="wsec")
                    nc.sync.dma_start(wvs[:], wv_d.rearrange("(k p) c -> p k c", p=128))
                    for tt in range(nt):
                        ps = pps.tile([128, 512], F32, name="vps", tag="psv")
                        for k in range(KC):
                            nc.tensor.matmul(
                                ps[:],
                                xTb[:, k, tt * 128:(tt + 1) * 128],
                                wvs[:, k, :],
                                start=(k == 0), stop=(k == KC - 1))
                        nc.vector.tensor_add(
                            vpm[:, tt].rearrange("p (h e) -> p h e", e=HD + 1)[:, :, 0:HD],
                            ps.rearrange("p (h e) -> p h e", e=HD),
                            bv_bc.rearrange("p (h e) -> p h e", e=HD))

            # -------- Phase B: attention (qi outer, heads inner) --------
            if "B" in phases:
              with tc.tile_pool(name="yT", bufs=1) as ypool:
                yT = [ypool.tile([128, t], F32R, name=f"yT{f}", tag=f"yT{f}")
                      for f in range(MQK)]

                with tc.tile_pool(name="esb", bufs=2) as pesb, \
                     tc.tile_pool(name="norm", bufs=2) as pnorm, \
                     tc.tile_pool(name="sc_ps", bufs=3, space="PSUM") as pscps, \
                     tc.tile_pool(name="y_ps", bufs=2, space="PSUM") as pyps:

                    for qi in range(nq):
                        nkc = 4 * qi + 4
                        for f in range(MQK):
                            for hh in range(2):
                                h = 2 * f + hh
                                qh = qkTb[:, f][hh * HD:(hh + 1) * HD, :]
                                kh = qkTb[:, MQK + f][hh * HD:(hh + 1) * HD, :]
                                # exp'd transposed scores for this q-chunk:
                                # esb[:, kc, :] = exp(k_tile_kc^T q_chunk)
                                esb = pesb.tile([128, nt, 512], BF16,
                                                name="esb", tag="esb")
                                y_acc = pyps.tile([HD + 1, 512], F32,
                                                  name=f"yacc{h}_{qi}",
                                                  tag="yacc")
                                qs = qh[:, qi * 512:(qi + 1) * 512]
                                for kc2 in range(nkc // 2):
                                    sp = pscps.tile([128, 2, 512], F32,
                                                    name="scps", tag="scps")
                                    for half in range(2):
                                        kc = 2 * kc2 + half
                                        # diagonal tiles: cols [0, 128r)
                                        # are fully masked; don't compute
                                        # them (attn-v skips them too, and
                                        # exp of the stale PSUM there is
                                        # finite and never consumed)
                                        r = kc - 4 * qi
                                        w0 = 128 * r if r > 0 else 0
                                        nc.tensor.matmul(
                                            sp[:, half, w0:],
                                            kh[:, kc * 128:(kc + 1) * 128],
                                            qs[:, w0:],
                                            start=True, stop=True)
                                    nc.scalar.activation(
                                        esb[:, 2 * kc2:2 * kc2 + 2, :],
                                        sp[:],
                                        AF.Copy if no_exp else AF.Exp,
                                        scale=float(SCALE))
                                    if kc2 >= 2 * qi and not no_mask:
                                        # diagonal pair: mask two triangle
                                        # blocks (cols (4qi+r)*512 + 128r)
                                        # in one strided op
                                        r0 = 2 * (kc2 - 2 * qi)
                                        diag = bass.AP(
                                            tensor=esb.tensor,
                                            offset=esb.offset
                                            + (4 * qi + r0) * 512 + 128 * r0,
                                            ap=[list(esb[:].ap[0]),
                                                [640, 2], [1, 128]])
                                        nc.vector.tensor_mul(
                                            diag, diag, tmask2[:])
                                for kc in range(nkc):
                                    # diagonal tiles: cols [0, 128r) are
                                    # fully masked (q < k) - skip them
                                    r = kc - 4 * qi
                                    w0 = 128 * r if r > 0 else 0
                                    nc.tensor.matmul(
                                        y_acc[:, w0:],
                                        vpm[:, kc, h * (HD + 1):(h + 1) * (HD + 1)],
                                        esb[:, kc, w0:],
                                        start=(kc == 0), stop=(kc == nkc - 1))
                                # normalize: yT slice = y/denom
                                if no_norm:
                                    nc.vector.tensor_copy(
                                        yT[f][hh * HD:(hh + 1) * HD,
                                              qi * 512:(qi + 1) * 512],
                                        y_acc[0:HD, :])
                                else:
                                    rec = pnorm.tile([1, 512], F32,
                                                     name="rec", tag="rec")
                                    nc.vector.reciprocal(
                                        rec[:], y_acc[HD:HD + 1, :])
                                    rb = pnorm.tile([HD, 512], F32,
                                                    name="rb", tag="rb")
                                    nc.gpsimd.partition_broadcast(rb[:], rec[:])
                                    nc.vector.tensor_mul(
                                        yT[f][hh * HD:(hh + 1) * HD,
                                              qi * 512:(qi + 1) * 512],
                                        y_acc[0:HD, :], rb[:])

                # ---------------- Phase C: projection ----------------
                if "C" in phases:
                  with tc.tile_pool(name="phC_sb", bufs=1) as pc, \
                       tc.tile_pool(name="phC_evac", bufs=3) as pcev, \
                       tc.tile_pool(name="phC_ps", bufs=3, space="PSUM") as pcps:
                    wpb = pc.tile([128, MQK, D], F32R)
                    nc.sync.dma_start(
                        wpb[:], wp_d.rearrange("(m p) o -> p m o", p=128))
                    for qtp in range(nt // 2):
                        ev = pcev.tile([128, 2, D], F32, name="prev", tag="prev")
                        for half in range(2):
                            qt = 2 * qtp + half
                            ps = pcps.tile([128, 1024], F32, name="prps", tag="prps")
                            for oc in range(D // 512):
                                for m in range(MQK):
                                    nc.tensor.matmul(
                                        ps[:, oc * 512:(oc + 1) * 512],
                                        yT[m][:, qt * 128:(qt + 1) * 128],
                                        wpb[:, m, oc * 512:(oc + 1) * 512],
                                        start=(m == 0), stop=(m == MQK - 1))
                            nc.vector.tensor_add(ev[:, half, :], ps[:], bp_bc[:])
                        nc.sync.dma_start(
                            out_d[qtp * 256:(qtp + 1) * 256, :]
                            .rearrange("(a p) o -> p a o", p=128),
                            ev[:])

    nc.finalize()
    return nc


def make_in_maps(x, w_attn, b_attn, w_proj, b_proj):
    x = np.ascontiguousarray(np.asarray(x, dtype=np.float32))
    w_attn = np.asarray(w_attn, dtype=np.float32)
    b_attn = np.asarray(b_attn, dtype=np.float32)
    w_proj = np.asarray(w_proj, dtype=np.float32)
    b_proj = np.asarray(b_proj, dtype=np.float32)
    in_maps = []
    for c in range(8):
        b, g = c // 2, c % 2
        sl = slice(DC * g, DC * (g + 1))
        in_maps.append({
            "xT": np.ascontiguousarray(x[b].T),
            "wq": np.ascontiguousarray(w_attn[:, 0 * D:][:, sl]),
            "wk": np.ascontiguousarray(w_attn[:, 1 * D:][:, sl]),
            "wv": np.ascontiguousarray(w_attn[:, 2 * D:][:, sl]),
            "bq": np.ascontiguousarray(b_attn[0 * D:1 * D][sl][None, :]),
            "bk": np.ascontiguousarray(b_attn[1 * D:2 * D][sl][None, :]),
            "bv": np.ascontiguousarray(b_attn[2 * D:3 * D][sl][None, :]),
            "wp": np.ascontiguousarray(w_proj[sl, :]),
            "bp": np.ascontiguousarray(
                (b_proj if g == 0 else np.zeros_like(b_proj))[None, :]),
            "cones": np.ones((1, 512), dtype=np.float32),
        })
    return in_maps


def kernel(x, w_attn, b_attn, w_proj, b_proj, _trace=False, _trace_kwargs=None):
    if "nc" not in _NC_CACHE:
        _NC_CACHE["nc"] = build_nc()
    nc = _NC_CACHE["nc"]
    in_maps = make_in_maps(x, w_attn, b_attn, w_proj, b_proj)
    kw = {}
    if _trace:
        kw["trace"] = True
        if _trace_kwargs:
            kw.update(_trace_kwargs)
    res = run_bass_kernel_spmd(nc, in_maps, core_ids=list(range(8)), **kw)
    outs = [res.results[c]["out"] for c in range(8)]
    out = np.empty((B, T, D), dtype=np.float32)
    for b in range(B):
        np.add(outs[2 * b], outs[2 * b + 1], out=out[b])
    kernel._last_results = res
    return out


if __name__ == "__main__":
    nc = build_nc()
    print("built ok")



# revision 25
# speedup vs baseline: 1.6426x; 1.6426x over previous
"""Causal self-attention Trainium2 Bass kernel.

Problem: B=4, T=2048, D=1024, H=16, head_dim=64.
Sharding: 8 cores = (batch b in 0..3) x (head-group g in 0..1, 8 heads each).
Each core computes a partial projection output for its batch over its 512
model dims; the host sums the two partials per batch (b_proj is fed to the
g==0 core only).

Kernel structure (per core) — chunk-pipelined over 512-col query chunks so
the Tensor engine stays dense and the Activation engine (exp, the phase-B
bottleneck) overlaps the projection phases:

- for n in 0..3:  A_qk(n) -> A_v(n) -> B(qi=n) -> C(n-1);  then C(3).
- A: qkv projections in bf16 (x and all weights are fed as bf16; PE rate is
  identical to fp32r, but DMA bytes halve and everything stays resident in
  SBUF so all phases can interleave).  q^T/k^T land in a resident bf16
  [feat, t] mega-tile; v natural-layout bf16 with a ones column per head
  (so attn @ v' also yields the softmax denominator).
- B: per head, transposed scores [keys, q] per key-tile pair go PSUM ->
  exp (ACT, scale=1/sqrt(hd), skipping fully-masked diagonal columns) ->
  bf16 pair tiles; causal diagonal blocks masked by one strided tensor_mul
  per pair; attn @ v' accumulates in a 1-bank PSUM tile shared (by tag)
  with C's projection PSUM.
- C: output projection from resident y^T bf16 tiles, bias fused into the
  PSUM evacuation, 2-row-group batched output DMAs on the sync queue.
- DMAs: weights first (wq sync / wk scalar / wv vector queues), x in
  column chunks on the gpsimd queue just ahead of use, so the first matmul
  starts ~3us in.

Timing note: `reps` is implemented as a device-side `tc.For_i` loop (the
body is idempotent, so executing it R times yields the same output).  A
repeat-delta between reps=1 and reps=R therefore measures the marginal
on-device execution time of one body iteration, with the fixed per-call
axon dispatch + NEFF transfer/load overhead cancelled.
"""

import contextlib

import ml_dtypes
import numpy as np

import concourse.bacc as bacc
import concourse.bass as bass
import concourse.mybir as mybir
import concourse.tile as tile
from concourse.bass_utils import run_bass_kernel_spmd

F32 = mybir.dt.float32
F32R = mybir.dt.float32r
BF16 = mybir.dt.bfloat16
AF = mybir.ActivationFunctionType

B, T, D, H = 4, 2048, 1024, 16
HD = 64              # head dim
HPC = 8              # heads per core
DC = HPC * HD        # 512 model dims per core
SCALE = 1.0 / np.sqrt(HD)

_NC_CACHE = {}


def build_nc(t=T, reps=1):
    """Build the single-core SPMD program. t = sequence length (for small
    sims). reps>1 repeats the computation via a device-side For_i loop
    (device-time measurement); reps=0 drops the loop (for local sims)."""
    nt = t // 128          # 128-row tiles over time
    nq = t // 512          # 512-col chunks over time
    KC = D // 128          # 8 contraction chunks for qkv
    MQK = DC // 128        # 4 feature tiles for each of q,k

    nc = bacc.Bacc("TRN2", target_bir_lowering=False, debug=False)

    xT_d = nc.dram_tensor("xT", [D, t], BF16, kind="ExternalInput")
    wq_d = nc.dram_tensor("wq", [D, DC], BF16, kind="ExternalInput")
    wk_d = nc.dram_tensor("wk", [D, DC], BF16, kind="ExternalInput")
    wv_d = nc.dram_tensor("wv", [D, DC], BF16, kind="ExternalInput")
    bq_d = nc.dram_tensor("bq", [1, DC], F32, kind="ExternalInput")
    bk_d = nc.dram_tensor("bk", [1, DC], F32, kind="ExternalInput")
    bv_d = nc.dram_tensor("bv", [1, DC], F32, kind="ExternalInput")
    wp_d = nc.dram_tensor("wp", [DC, D], BF16, kind="ExternalInput")
    bp_d = nc.dram_tensor("bp", [1, D], F32, kind="ExternalInput")
    ones_d = nc.dram_tensor("cones", [1, 512], BF16, kind="ExternalInput")
    out_d = nc.dram_tensor("out", [t, D], F32, kind="ExternalOutput")

    with tile.TileContext(nc) as tc:
      with (tc.For_i(0, reps, 1) if reps else contextlib.nullcontext()):
        with tc.tile_pool(name="persist", bufs=1) as persist, \
             tc.tile_pool(name="wpool", bufs=1) as wpool, \
             tc.tile_pool(name="qkpool", bufs=1) as qkpool, \
             tc.tile_pool(name="vpool", bufs=1) as vpool, \
             tc.tile_pool(name="ypool", bufs=1) as ypool, \
             tc.tile_pool(name="xpool", bufs=2) as xpool, \
             tc.tile_pool(name="epool", bufs=6) as epool, \
             tc.tile_pool(name="evpool", bufs=2) as evpool, \
             tc.tile_pool(name="npool", bufs=2) as npool, \
             tc.tile_pool(name="aux_ps", bufs=2, space="PSUM") as auxps, \
             tc.tile_pool(name="sc_ps", bufs=2, space="PSUM") as scps, \
             tc.tile_pool(name="yc_ps", bufs=2, space="PSUM") as ycps:

            # ---- weight tiles; wq DMA first (gates the first matmul),
            # wk/wv/wpb DMAs are emitted inside the n-loop so the x-chunk
            # stream isn't stuck behind them on the queues ----
            wqs = wpool.tile([128, KC, DC], BF16, name="wqs", tag="wqs")
            # first half (m-tiles 0,1) gates the first qk chain; second half
            # is emitted after x(0)'s sync-queue quarters below
            nc.sync.dma_start(
                wqs[:, :, 0:256],
                wq_d[:, 0:256].rearrange("(k p) c -> p k c", p=128))
            wks = wpool.tile([128, KC, DC], BF16, name="wks", tag="wks")
            wvs = wpool.tile([128, KC, DC], BF16, name="wvs", tag="wvs")
            wpb = wpool.tile([128, MQK, D], BF16, name="wpb", tag="wpb")

            # ---- small persistent tiles ----
            # tmask[p, c] = 1 iff c >= p (keep lower triangle in [k, q])
            tmask = persist.tile([128, 128], BF16)
            nc.gpsimd.memset(tmask[:], 1.0)
            nc.gpsimd.affine_select(
                out=tmask[:], in_=tmask[:],
                compare_op=mybir.AluOpType.is_ge, fill=0.0,
                base=0, pattern=[[1, 128]], channel_multiplier=-1)
            # tmask2: 2 replicas so one strided mul masks a diagonal pair
            tmask2 = persist.tile([128, 2, 128], BF16)
            for r in range(2):
                nc.vector.tensor_copy(tmask2[:, r, :], tmask[:])

            # partition-major per-feature-tile bias columns [128, MQK]
            # (first: they gate the first qk evacuations)
            bqp = persist.tile([128, MQK], F32)
            nc.gpsimd.dma_start(bqp[:], bq_d.rearrange("o (m p) -> p (o m)", p=128))
            bkp = persist.tile([128, MQK], F32)
            nc.gpsimd.dma_start(bkp[:], bk_d.rearrange("o (m p) -> p (o m)", p=128))
            bv_bc = persist.tile([128, DC], F32)
            nc.gpsimd.dma_start(bv_bc[:], bv_d[0:1, :].to_broadcast([128, DC]))
            ones_bc = persist.tile([128, nt * HPC], BF16)
            nc.gpsimd.dma_start(
                ones_bc[:], ones_d[0:1, 0:nt * HPC].to_broadcast([128, nt * HPC]))
            bp_bc = persist.tile([128, D], F32)
            nc.gpsimd.dma_start(bp_bc[:], bp_d[0:1, :].to_broadcast([128, D]))

            # ---- resident outputs of phase A ----
            # qkTb[:, m, :] = q^T feats tile m; [:, MQK+m, :] = k^T
            qkTb = qkpool.tile([128, 2 * MQK, t], BF16)
            # v' mega-tile (bf16): [128, nt, 8*65]; col h*65+64 holds ones
            vpm = vpool.tile([128, nt, HPC * (HD + 1)], BF16)
            nc.vector.tensor_copy(
                vpm.rearrange("p t (h e) -> p (t h) e", e=HD + 1)[:, :, HD:HD + 1],
                ones_bc[:].unsqueeze(2))
            yT = [ypool.tile([128, t], BF16, name=f"yT{f}", tag=f"yT{f}")
                  for f in range(MQK)]

            def emit_C(j, fine=False):
                """Projection for query rows [j*512, (j+1)*512).  fine=True
                emits per-row-tile output DMAs on alternating queues (for
                the last chunk: shortens the drain tail)."""
                ngrp, rows = (4, 1) if fine else (2, 2)
                for g in range(ngrp):
                    ev = evpool.tile([128, rows, D], F32, name="prev",
                                     tag="prev")
                    for half in range(rows):
                        qt = 4 * j + rows * g + half
                        for oc in range(2):
                            cps = ycps.tile([128, 512], F32, name="cps",
                                            tag="yc")
                            for m in range(MQK):
                                nc.tensor.matmul(
                                    cps[:],
                                    yT[m][:, qt * 128:(qt + 1) * 128],
                                    wpb[:, m, oc * 512:(oc + 1) * 512],
                                    start=(m == 0), stop=(m == MQK - 1))
                            nc.vector.tensor_add(
                                ev[:, half, oc * 512:(oc + 1) * 512],
                                cps[:], bp_bc[:, oc * 512:(oc + 1) * 512])
                    q_eng = nc.sync if g % 2 == 0 else nc.scalar
                    r0 = (4 * j + rows * g) * 128
                    q_eng.dma_start(
                        out_d[r0:r0 + rows * 128, :]
                        .rearrange("(a p) o -> p a o", p=128),
                        ev[:])

            def emit_A(n):
                # x column-chunk for this n: one HWDGE DMA per half,
                # spread over the scalar and sync queues
                xc = xpool.tile([128, KC, 512], BF16, name="xc", tag="xc")
                if n == 0:
                    # quarters on alternating queues: minimizes the time to
                    # the first matmul of the kernel
                    for qtr in range(4):
                        q_eng = nc.scalar if qtr < 2 else nc.sync
                        q_eng.dma_start(
                            xc[:, 2 * qtr:2 * qtr + 2, :],
                            xT_d[256 * qtr:256 * (qtr + 1),
                                 n * 512:(n + 1) * 512]
                            .rearrange("(k p) c -> p k c", p=128))
                    nc.sync.dma_start(
                        wqs[:, :, 256:512],
                        wq_d[:, 256:512].rearrange("(k p) c -> p k c", p=128))
                else:
                    nc.scalar.dma_start(
                        xc[:, 0:KC // 2, :],
                        xT_d[0:512, n * 512:(n + 1) * 512]
                        .rearrange("(k p) c -> p k c", p=128))
                    nc.sync.dma_start(
                        xc[:, KC // 2:KC, :],
                        xT_d[512:1024, n * 512:(n + 1) * 512]
                        .rearrange("(k p) c -> p k c", p=128))
                if n == 0:
                    nc.scalar.dma_start(
                        wks[:], wk_d.rearrange("(k p) c -> p k c", p=128))
                    nc.scalar.dma_start(
                        wvs[:], wv_d.rearrange("(k p) c -> p k c", p=128))
                if n == 1:
                    nc.sync.dma_start(
                        wpb[:], wp_d.rearrange("(m p) o -> p m o", p=128))

                # ---- A_qk(n): q^T/k^T feature tiles for time cols n,
                # interleaved (q_m, k_m) so head-pair f of B(n) is
                # unblocked after just two chains; q evacuates on ACT,
                # k on DVE so the two gates run in parallel ----
                for m in range(MQK):
                    for sec, (w_s, b_s) in enumerate(((wqs, bqp),
                                                      (wks, bkp))):
                        ps = auxps.tile([128, 512], F32, name="aps",
                                        tag="aps")
                        for kc in range(KC):
                            nc.tensor.matmul(
                                ps[:],
                                w_s[:, kc, m * 128:(m + 1) * 128],
                                xc[:, kc, :],
                                start=(kc == 0), stop=(kc == KC - 1))
                        if sec == 0:
                            # Identity(in + per-partition bias); same table
                            # set as Exp, so no ACT_TABLE_LOAD thrash
                            nc.scalar.activation(
                                qkTb[:, m, n * 512:(n + 1) * 512],
                                ps[:], AF.Identity, bias=b_s[:, m:m + 1])
                        else:
                            nc.vector.tensor_scalar_add(
                                qkTb[:, MQK + m, n * 512:(n + 1) * 512],
                                ps[:], b_s[:, m:m + 1])
                return xc

            def emit_V(n, xc):
                # ---- A_v(n): natural-layout v tiles 4n..4n+3 ----
                for ti in range(4):
                    tt = 4 * n + ti
                    ps = auxps.tile([128, 512], F32, name="aps", tag="aps")
                    for kc in range(KC):
                        nc.tensor.matmul(
                            ps[:],
                            xc[:, kc, ti * 128:(ti + 1) * 128],
                            wvs[:, kc, :],
                            start=(kc == 0), stop=(kc == KC - 1))
                    nc.vector.tensor_add(
                        vpm[:, tt].rearrange(
                            "p (h e) -> p h e", e=HD + 1)[:, :, 0:HD],
                        ps[:].rearrange("p (h e) -> p h e", e=HD),
                        bv_bc.rearrange("p (h e) -> p h e", e=HD))

            def emit_B(n, after_first_head=None):
                # ---- B(qi = n): attention for query cols n ----
                nkc = 4 * n + 4
                for f in range(MQK):
                    for hh in range(2):
                        if after_first_head is not None and f + hh == 1:
                            # v(n) matmuls emitted below the first head's
                            # scores in priority, above everything later
                            after_first_head()
                            after_first_head = None
                        h = 2 * f + hh
                        qh = qkTb[:, f][hh * HD:(hh + 1) * HD, :]
                        kh = qkTb[:, MQK + f][hh * HD:(hh + 1) * HD, :]
                        qs = qh[:, n * 512:(n + 1) * 512]
                        eps = []
                        for kc2 in range(nkc // 2):
                            sp = scps.tile([128, 2, 512], F32, name="sp",
                                           tag="sp")
                            for halfk in range(2):
                                kc = 2 * kc2 + halfk
                                # diagonal tiles: cols [0, 128r) are fully
                                # masked; don't compute them
                                r = kc - 4 * n
                                w0 = 128 * r if r > 0 else 0
                                nc.tensor.matmul(
                                    sp[:, halfk, w0:],
                                    kh[:, kc * 128:(kc + 1) * 128],
                                    qs[:, w0:],
                                    start=True, stop=True)
                            ep = epool.tile([128, 2, 512], BF16, name="ep",
                                            tag="ep")
                            eps.append(ep)
                            diag = kc2 >= 2 * n
                            # both tiles of a diagonal pair have cols
                            # [0, 128*r0) fully masked: skip them in exp too
                            r0 = 2 * (kc2 - 2 * n) if diag else 0
                            w0p = 128 * r0
                            nc.scalar.activation(
                                ep[:, :, w0p:], sp[:, :, w0p:],
                                AF.Exp, scale=float(SCALE))
                            if diag:
                                # mask the two 128x128 triangle blocks of
                                # this pair in one strided op
                                dap = bass.AP(
                                    tensor=ep.tensor,
                                    offset=ep.offset + 128 * r0,
                                    ap=[list(ep[:].ap[0]),
                                        [640, 2], [1, 128]])
                                nc.vector.tensor_mul(dap, dap, tmask2[:])
                        y_acc = ycps.tile([HD + 1, 512], F32,
                                          name=f"yacc{h}_{n}", tag="yc")
                        for kc in range(nkc):
                            r = kc - 4 * n
                            w0 = 128 * r if r > 0 else 0
                            nc.tensor.matmul(
                                y_acc[:, w0:],
                                vpm[:, kc, h * (HD + 1):(h + 1) * (HD + 1)],
                                eps[kc // 2][:, kc % 2, w0:],
                                start=(kc == 0), stop=(kc == nkc - 1))
                        # normalize: yT slice = y / denom
                        rec = npool.tile([1, 512], F32, name="rec", tag="rec")
                        nc.vector.reciprocal(rec[:], y_acc[HD:HD + 1, :])
                        rb = npool.tile([HD, 512], F32, name="rb", tag="rb")
                        nc.gpsimd.partition_broadcast(rb[:], rec[:])
                        nc.vector.tensor_mul(
                            yT[f][hh * HD:(hh + 1) * HD,
                                  n * 512:(n + 1) * 512],
                            y_acc[0:HD, :], rb[:])

            # emission order sets scheduler priority
            for n in range(nq):
                xc = emit_A(n)
                if n > 0:
                    emit_C(n - 1)
                emit_B(n, after_first_head=lambda xc=xc, n=n: emit_V(n, xc))
            emit_C(nq - 1, fine=True)

    nc.finalize()
    return nc


def make_in_maps(x, w_attn, b_attn, w_proj, b_proj):
    bf16 = ml_dtypes.bfloat16
    x = np.asarray(x, dtype=np.float32)
    w_attn = np.asarray(w_attn, dtype=np.float32)
    b_attn = np.asarray(b_attn, dtype=np.float32)
    w_proj = np.asarray(w_proj, dtype=np.float32)
    b_proj = np.asarray(b_proj, dtype=np.float32)
    in_maps = []
    for c in range(8):
        b, g = c // 2, c % 2
        sl = slice(DC * g, DC * (g + 1))
        in_maps.append({
            "xT": np.ascontiguousarray(x[b].T).astype(bf16),
            "wq": np.ascontiguousarray(w_attn[:, 0 * D:][:, sl]).astype(bf16),
            "wk": np.ascontiguousarray(w_attn[:, 1 * D:][:, sl]).astype(bf16),
            "wv": np.ascontiguousarray(w_attn[:, 2 * D:][:, sl]).astype(bf16),
            "bq": np.ascontiguousarray(b_attn[0 * D:1 * D][sl][None, :]),
            "bk": np.ascontiguousarray(b_attn[1 * D:2 * D][sl][None, :]),
            "bv": np.ascontiguousarray(b_attn[2 * D:3 * D][sl][None, :]),
            "wp": np.ascontiguousarray(w_proj[sl, :]).astype(bf16),
            "bp": np.ascontiguousarray(
                (b_proj if g == 0 else np.zeros_like(b_proj))[None, :]),
            "cones": np.ones((1, 512), dtype=bf16),
        })
    return in_maps


def kernel(x, w_attn, b_attn, w_proj, b_proj, _trace=False, _trace_kwargs=None):
    if "nc" not in _NC_CACHE:
        _NC_CACHE["nc"] = build_nc()
    nc = _NC_CACHE["nc"]
    in_maps = make_in_maps(x, w_attn, b_attn, w_proj, b_proj)
    kw = {}
    if _trace:
        kw["trace"] = True
        if _trace_kwargs:
            kw.update(_trace_kwargs)
    res = run_bass_kernel_spmd(nc, in_maps, core_ids=list(range(8)), **kw)
    outs = [res.results[c]["out"] for c in range(8)]
    out = np.empty((B, T, D), dtype=np.float32)
    for b in range(B):
        np.add(outs[2 * b], outs[2 * b + 1], out=out[b])
    kernel._last_results = res
    return out


if __name__ == "__main__":
    nc = build_nc()
    print("built ok")
